# revision 5
# baseline (speedup 1.0000x reference)
"""CompressedLinear Trainium2 kernel — one-level Strassen variant.

out[b,s,o] = x[b,s,i] @ (int8_w[o,i] * scale).T + bias[o]
x: [4,2048,4096] f32, w: [11008,4096] int32 (int8 vals), scale f32, bias [11008].

Sharding: column-parallel over 8 cores (1376 out-features each), x replicated.

Per core, one Strassen level over [S=8192, K=4096] @ [K, O=1376]:
  S split: S1 = rows [0,4096), S2 = [4096,8192)
  K split: K1 = [0,2048), K2 = [2048,4096)
  O split: O1 = [0,688), O2 = [688,1376)
  7 products Mi = Ai @ Bi with [4096 x 2048] @ [2048 x 688]  (7/8 the MACs)
    A1=X11+X22 B1=W11+W22 | A2=X21+X22 B2=W11 | A3=X11 B3=W12-W22
    A4=X22 B4=W21-W11     | A5=X11+X12 B5=W22 | A6=X21-X11 B6=W11+W12
    A7=X12-X22 B7=W21+W22
  O11=M1+M4-M5+M7  O12=M3+M5  O21=M2+M4  O22=M1-M2+M3+M6
All A/B combos are computed on host. B combos are integers |.|<=254 — exact
in bf16. Device: 7 psum banks accumulate the Mi per (s-block, o-chunk);
DVE combines + scale/bias epilogue; DMA stores the four output quadrants.
"""

import numpy as np
import ml_dtypes

import concourse.bacc as bacc
import concourse.mybir as mybir
import concourse.tile as tile
from concourse.bass_utils import run_bass_kernel_spmd

B, S, IN_F, OUT_F = 4, 2048, 4096, 11008
NCORES = 8
OUT_PER = OUT_F // NCORES  # 1376
S_TOT = B * S  # 8192

SV = S_TOT // 2  # 4096 virtual rows
KH = IN_F // 2  # 2048
OH = OUT_PER // 2  # 688
KT = KH // 128  # 16 k-tiles
NVSB = SV // 128  # 32 virtual s-blocks
CPS = [(0, 512), (512, 176)]  # o-chunks within a 688-wide half

bf16np = ml_dtypes.bfloat16

TRACE = False
LAST_RESULT = None

_cache = {}


def build_nc():
    f32 = mybir.dt.float32
    bf16 = mybir.dt.bfloat16

    nc = bacc.Bacc("TRN2", target_bir_lowering=False, debug=False, num_devices=NCORES)

    a_d = [
        nc.dram_tensor(f"a{i}", [NVSB * 128, KT * 128], bf16, kind="ExternalInput").ap()
        for i in range(7)
    ]
    b_d = {
        (i, h): nc.dram_tensor(
            f"b{i}h{h}", [4 * 128, 4 * CPS[h][1]], bf16, kind="ExternalInput"
        ).ap()
        for i in range(7)
        for h in range(2)
    }
    bias = nc.dram_tensor("bias", [1, OUT_PER], f32, kind="ExternalInput").ap()
    scale = nc.dram_tensor("scale", [1, 1], f32, kind="ExternalInput").ap()
    out = nc.dram_tensor("out", [S_TOT, OUT_PER], f32, kind="ExternalOutput").ap()

    with tile.TileContext(nc) as tc:
        with (
            tc.tile_pool(name="wt", bufs=1) as wt_pool,
            tc.tile_pool(name="abf", bufs=8) as a_pool,
            tc.tile_pool(name="psum", bufs=1, space="PSUM") as psum_pool,
            tc.tile_pool(name="osb", bufs=2) as osb_pool,
            tc.tile_pool(name="consts", bufs=1) as const_pool,
        ):
            # HAM warmup: PE clock-gate ramp (4/8 cold -> 8/8 after ~3.4us).
            zeros = const_pool.tile([128, 512], bf16, tag="zeros", name="zeros")
            nc.vector.memset(zeros[:], 0)
            psw = psum_pool.tile([128, 512], f32, tag="warm", name="warm")
            for i in range(16):
                nc.tensor.matmul(
                    psw[:, :], zeros[:, 0:128], zeros[:, :], start=True, stop=True
                )
            for i in range(44):
                nc.tensor.matmul(
                    psw[:, 0:128], zeros[:, 0:128], zeros[:, 0:128],
                    start=True, stop=True,
                )

            # W' combos: per product, per k-group, per o-chunk-half tiles
            # [128, 4, cpw] bf16. Load order: (h0,g0) -> vsb0's A tiles ->
            # rest of h0 -> h1, so the PE starts real matmuls ~4us in and
            # is never gated on the full 19.7 MB.
            def emit_w(g, h):
                cp0, cpw = CPS[h]
                for i in range(7):
                    t = wt_pool.tile(
                        [128, 4, cpw], bf16, tag=f"w{i}g{g}h{h}", name=f"w{i}g{g}h{h}"
                    )
                    src = b_d[(i, h)][g * 128 : (g + 1) * 128, :].rearrange(
                        "p (g o) -> p g o", g=4
                    )
                    nc.sync.dma_start(out=t[:], in_=src)
                    wt_tiles[(i, g, h)] = t

            wt_tiles = {}
            emit_w(0, 0)

            # vsb0 A tiles, hoisted ahead of the remaining W' loads
            a_t_v0 = []
            for i in range(7):
                t = a_pool.tile([128, KT, 128], bf16, tag="a", name=f"a{i}_0")
                src = a_d[i][0:128, :].rearrange("p (g s) -> p g s", g=KT)
                nc.sync.dma_start(out=t[:], in_=src)
                a_t_v0.append(t)

            for g in range(1, 4):
                emit_w(g, 0)
            for g in range(4):
                emit_w(g, 1)

            scale_sb = const_pool.tile([128, 1], f32, tag="scale", name="scale_sb")
            nc.sync.dma_start(out=scale_sb[:], in_=scale.partition_broadcast(128))
            bias_sb = const_pool.tile([128, OUT_PER], f32, tag="bias", name="bias_sb")
            nc.sync.dma_start(out=bias_sb[:], in_=bias.partition_broadcast(128))

            add = mybir.AluOpType.add
            sub = mybir.AluOpType.subtract
            mult = mybir.AluOpType.mult

            for v in range(NVSB):
                s0 = v * 128
                # A tiles for this vsb: [128, 16, 128] bf16 per product
                if v == 0:
                    a_t = a_t_v0
                else:
                    a_t = []
                    for i in range(7):
                        t = a_pool.tile([128, KT, 128], bf16, tag="a", name=f"a{i}_{v}")
                        src = a_d[i][s0 : s0 + 128, :].rearrange(
                            "p (g s) -> p g s", g=KT
                        )
                        nc.sync.dma_start(out=t[:], in_=src)
                        a_t.append(t)

                for h, (cp0, cpw) in enumerate(CPS):
                    ps = [
                        psum_pool.tile([128, 512], f32, tag=f"m{i}", name=f"m{i}_{v}_{cp0}")
                        for i in range(7)
                    ]
                    # vsb0: k-group-major so first MMs chase the W' load
                    # stream. Later vsbs: product-major so each A tile's
                    # last read lands early, widening the prefetch window.
                    if v == 0:
                        order = [
                            (i, k)
                            for g in range(4)
                            for i in range(7)
                            for k in range(g * 4, (g + 1) * 4)
                        ]
                    else:
                        order = [(i, k) for i in range(7) for k in range(KT)]
                    # during the W'-load chase (first vsbs), pad each
                    # 4-MM run with short no-dep filler MMs: the in-order PE
                    # queue executes them where it would otherwise idle on
                    # DMA, keeping the clock ramp (P-state) from resetting.
                    filler = {0: 3, 1: 2, 2: 1}.get(v, 0)
                    for n_mm, (i, k) in enumerate(order):
                        g = k // 4
                        wg = wt_tiles[(i, g, h)]
                        nc.tensor.matmul(
                            ps[i][:, :cpw],
                            a_t[i][:, k, :],
                            wg[:, k - g * 4, :cpw],
                            start=(k == 0),
                            stop=(k == KT - 1),
                        )
                        if filler and n_mm % 4 == 3:
                            for _ in range(filler):
                                nc.tensor.matmul(
                                    psw[:, 0:128], zeros[:, 0:128],
                                    zeros[:, 0:128], start=True, stop=True,
                                )

                    # combines + epilogue; column ranges: O1-half = cp0,
                    # O2-half = 688+cp0. Row ranges: S1 = s0, S2 = 4096+s0.
                    c1 = cp0
                    c2 = OH + cp0

                    # DVE reads at most one PSUM operand per op: copy first,
                    # then chain single-psum adds, then scale+bias.
                    o21 = osb_pool.tile([128, 512], f32, tag="o21", name=f"o21_{v}_{cp0}")
                    nc.vector.tensor_copy(o21[:, :cpw], ps[1][:, :cpw])
                    nc.vector.tensor_tensor(o21[:, :cpw], o21[:, :cpw], ps[3][:, :cpw], add)
                    nc.vector.scalar_tensor_tensor(
                        o21[:, :cpw], o21[:, :cpw], scale_sb[:, 0:1],
                        bias_sb[:, c1 : c1 + cpw], mult, add,
                    )
                    nc.sync.dma_start(
                        out=out[SV + s0 : SV + s0 + 128, c1 : c1 + cpw], in_=o21[:, :cpw]
                    )

                    o12 = osb_pool.tile([128, 512], f32, tag="o12", name=f"o12_{v}_{cp0}")
                    nc.vector.tensor_copy(o12[:, :cpw], ps[2][:, :cpw])
                    nc.vector.tensor_tensor(o12[:, :cpw], o12[:, :cpw], ps[4][:, :cpw], add)
                    nc.vector.scalar_tensor_tensor(
                        o12[:, :cpw], o12[:, :cpw], scale_sb[:, 0:1],
                        bias_sb[:, c2 : c2 + cpw], mult, add,
                    )
                    nc.sync.dma_start(
                        out=out[s0 : s0 + 128, c2 : c2 + cpw], in_=o12[:, :cpw]
                    )

                    o11 = osb_pool.tile([128, 512], f32, tag="o11", name=f"o11_{v}_{cp0}")
                    nc.vector.tensor_copy(o11[:, :cpw], ps[0][:, :cpw])
                    nc.vector.tensor_tensor(o11[:, :cpw], o11[:, :cpw], ps[3][:, :cpw], add)
                    nc.vector.tensor_tensor(o11[:, :cpw], o11[:, :cpw], ps[4][:, :cpw], sub)
                    nc.vector.tensor_tensor(o11[:, :cpw], o11[:, :cpw], ps[6][:, :cpw], add)
                    nc.vector.scalar_tensor_tensor(
                        o11[:, :cpw], o11[:, :cpw], scale_sb[:, 0:1],
                        bias_sb[:, c1 : c1 + cpw], mult, add,
                    )
                    nc.sync.dma_start(
                        out=out[s0 : s0 + 128, c1 : c1 + cpw], in_=o11[:, :cpw]
                    )

                    o22 = osb_pool.tile([128, 512], f32, tag="o22", name=f"o22_{v}_{cp0}")
                    nc.vector.tensor_copy(o22[:, :cpw], ps[0][:, :cpw])
                    nc.vector.tensor_tensor(o22[:, :cpw], o22[:, :cpw], ps[1][:, :cpw], sub)
                    nc.vector.tensor_tensor(o22[:, :cpw], o22[:, :cpw], ps[2][:, :cpw], add)
                    nc.vector.tensor_tensor(o22[:, :cpw], o22[:, :cpw], ps[5][:, :cpw], add)
                    nc.vector.scalar_tensor_tensor(
                        o22[:, :cpw], o22[:, :cpw], scale_sb[:, 0:1],
                        bias_sb[:, c2 : c2 + cpw], mult, add,
                    )
                    nc.sync.dma_start(
                        out=out[SV + s0 : SV + s0 + 128, c2 : c2 + cpw], in_=o22[:, :cpw]
                    )

    nc.compile()
    return nc


def _get_nc():
    if "s" not in _cache:
        _cache["s"] = build_nc()
    return _cache["s"]


def kernel(x, weight_int8, scale, bias):
    global LAST_RESULT
    x = np.asarray(x, dtype=np.float32)
    w = np.asarray(weight_int8)
    scale_f = np.float32(np.asarray(scale).reshape(()))
    bias = np.asarray(bias, dtype=np.float32)

    xf = x.reshape(S_TOT, IN_F)
    X11 = xf[:SV, :KH]
    X12 = xf[:SV, KH:]
    X21 = xf[SV:, :KH]
    X22 = xf[SV:, KH:]
    a_list = [
        X11 + X22, X21 + X22, X11, X22, X11 + X12, X21 - X11, X12 - X22,
    ]
    # [k, s] bf16, contiguous
    a_np = {}
    for i, a in enumerate(a_list):
        at = a.T.reshape(KT, 128, NVSB, 128)  # [g, p, v, s]
        arr = at.transpose(2, 1, 0, 3).reshape(NVSB * 128, KT * 128)
        a_np[f"a{i}"] = np.ascontiguousarray(arr).astype(bf16np)

    wf = w.astype(np.float32)  # [out, in]
    scale_rep = np.full((1, 1), scale_f, dtype=np.float32)

    nc = _get_nc()
    in_maps = []
    for c in range(NCORES):
        o0 = c * OUT_PER
        wc = wf[o0 : o0 + OUT_PER, :].T  # [in, out_per]
        W11 = wc[:KH, :OH]
        W12 = wc[:KH, OH:]
        W21 = wc[KH:, :OH]
        W22 = wc[KH:, OH:]
        b_list = [
            W11 + W22, W11, W12 - W22, W21 - W11, W22, W11 + W12, W21 + W22,
        ]
        m = {}
        for i, b in enumerate(b_list):
            b4 = b.reshape(4, 4, 128, OH)  # [g, gg, p, o]
            for h, (cp0, cpw) in enumerate(CPS):
                arr = b4[:, :, :, cp0 : cp0 + cpw].transpose(0, 2, 1, 3)
                m[f"b{i}h{h}"] = np.ascontiguousarray(
                    arr.reshape(4 * 128, 4 * cpw)
                ).astype(bf16np)
        m.update(a_np)
        m["bias"] = np.ascontiguousarray(bias[o0 : o0 + OUT_PER][None, :])
        m["scale"] = scale_rep
        in_maps.append(m)

    res = run_bass_kernel_spmd(nc, in_maps, core_ids=list(range(NCORES)), trace=TRACE)
    LAST_RESULT = res
    out = np.concatenate([res.results[c]["out"] for c in range(NCORES)], axis=1)
    return out.reshape(B, S, OUT_F)
